# revision 1
# baseline (speedup 1.0000x reference)
"""GTN (graph transformer network) meta-path kernel for TRN2, 8 NeuronCores.

Math (reference):
    Ap = A transposed to [E, N, N]
    a  = sum_e softmax(w1_0)[c,e] * Ap[e]      (per channel c)
    b  = sum_e softmax(w2_0)[c,e] * Ap[e]
    H  = a @ b
    twice:  H = normalize(H) @ gtconv(Ap, w)   (normalize = zero diag, col-scale)
    out = symmetrized mean over channels.

Sharding: channel-parallel — core c computes channel c end to end (the four
softmax mixes differ only in the tiny [E] weight vector, fed per-core), then
one AllReduce over the 8 cores and a local symmetrization.

On-device formulation works with TRANSPOSED intermediates Ht = H^T so that
 - every GEMM's moving operand is the previous GEMM's output as-is,
 - normalization becomes row sums (free-dim reduce) + per-partition scale.

The edge-type mix contracts e on SBUF partitions, so the host feeds A once
in a pre-permuted bf16 layout At3[b, (k16 e), j]; one PE pass with a
block-diagonal [128, 64] weight computes all four mixes in natural
orientation, written to DRAM in the PSUM-packed row order (row = kb*64 +
q*16 + k16).  GEMM stationaries load straight from that packed layout with
multi-dim APs; the 'a' mix is unpacked once (DRAM->DRAM) and transposed on
the PE into the GEMM1 moving operand.  HWDGE DMA issue costs ~0.6 us of
sequencer time each, so the design minimizes DMA instruction count.
"""

import numpy as np

N = 2048
E = 8
C = 8
P = 128
NCORES = 8

_PROGRAM = None


def _softmax_rows(w: np.ndarray) -> np.ndarray:
    """w: [C, E, 1, 1] -> softmax over E, float64 precision, returns [C, E]."""
    x = w.reshape(C, E).astype(np.float64)
    x = x - x.max(axis=1, keepdims=True)
    ex = np.exp(x)
    return ex / ex.sum(axis=1, keepdims=True)


def _build_program():
    import concourse.bacc as bacc
    import concourse.mybir as mybir
    import concourse.tile as tile
    from concourse.masks import make_identity

    f32 = mybir.dt.float32
    bf16 = mybir.dt.bfloat16
    AX = mybir.AxisListType.X
    MUL = mybir.AluOpType.mult
    ADD = mybir.AluOpType.add
    NE = mybir.AluOpType.not_equal
    COPY = mybir.ActivationFunctionType.Copy

    nc = bacc.Bacc("TRN2")
    A3_ext = nc.dram_tensor("At3", [P, P, N], bf16, kind="ExternalInput")
    w4_ext = nc.dram_tensor("wblk4", [P, 64], bf16, kind="ExternalInput")
    out_ext = nc.dram_tensor("out", [N, N], f32, kind="ExternalOutput")

    with tile.TileContext(nc) as tc:
        with (
            tc.tile_pool(name="dram", bufs=1, space="DRAM") as dpool,
            tc.tile_pool(name="const", bufs=1) as cpool,
        ):
            # all four mixes, psum-packed: row = kb*64 + q*16 + k16
            # quartered so unpacking can start before the whole mix finishes
            packed = [
                dpool.tile([N, N], bf16, name=f"packed{qt}") for qt in range(4)
            ]
            anat = dpool.tile([N, N], bf16)         # a in natural [i, kappa]
            nat = [dpool.tile([N, N], bf16, name=f"nat{q}") for q in range(1, 4)]
            # per-channel H''^T and allreduced sum, in 4 row bands so
            # collectives pipeline with GEMM3 and phase 6
            h2t = [dpool.tile([512, N], f32, name=f"h2t{b}") for b in range(4)]
            s_sh = [
                dpool.tile([512, N], f32, addr_space="Shared", name=f"ssh{b}")
                for b in range(4)
            ]

            # --- constants ---
            w4_sb = cpool.tile([P, 64], bf16)
            nc.sync.dma_start(out=w4_sb[:], in_=w4_ext[:])
            ident = cpool.tile([P, P], f32)
            make_identity(nc, ident[:])
            identb = cpool.tile([P, P], bf16)
            make_identity(nc, identb[:])
            # diag masks: masks[:, v, y] = 0 where y == p + v*128 else 1
            masks = cpool.tile([P, 4, 512], f32)
            nc.gpsimd.memset(masks[:], 1.0)
            for v in range(4):
                nc.gpsimd.affine_select(
                    out=masks[:, v],
                    in_=masks[:, v],
                    compare_op=NE,
                    fill=0.0,
                    base=v * P,
                    pattern=[[-1, 512]],
                    channel_multiplier=1,
                )

            # =========== Phase 1: all four mixes in one PE pass ===========
            with (
                tc.tile_pool(name="mix", bufs=3) as mpool,
                tc.tile_pool(name="mixst", bufs=8) as spool,
                tc.tile_pool(name="mixps", bufs=6, space="PSUM") as mpsum,
            ):
                for ld4 in range(32):
                    a3t = mpool.tile([P, 4, N], bf16, tag="a3t")
                    nc.sync.dma_start(
                        out=a3t[:],
                        in_=A3_ext[4 * ld4 : 4 * ld4 + 4].rearrange(
                            "b p j -> p b j"
                        ),
                    )
                    for half in range(2):
                        bp = ld4 * 2 + half
                        qt, bpl = bp // 16, bp % 16
                        for jc in range(4):
                            pm = mpsum.tile([P, 512], f32, tag="pm")
                            for h in range(2):
                                nc.tensor.matmul(
                                    pm[h * 64 : (h + 1) * 64, :],
                                    lhsT=w4_sb[:],
                                    rhs=a3t[
                                        :,
                                        half * 2 + h,
                                        jc * 512 : (jc + 1) * 512,
                                    ],
                                    start=True,
                                    stop=True,
                                )
                            st = spool.tile([P, 512], bf16, tag="st")
                            if jc % 2 == 0:
                                nc.vector.tensor_copy(out=st[:], in_=pm[:])
                            else:
                                nc.scalar.copy(st[:], pm[:])
                            weng = nc.scalar if jc % 2 == 0 else nc.sync
                            weng.dma_start(
                                out=packed[qt][
                                    bpl * P : (bpl + 1) * P,
                                    jc * 512 : (jc + 1) * 512,
                                ],
                                in_=st[:],
                            )
                    if ld4 % 8 == 7:
                        # this quarter of packed is complete: unpack (d2d)
                        qt = ld4 // 8
                        pk5 = packed[qt][:].rearrange(
                            "(bp h q k) j -> bp h q k j", h=2, q=4, k=16
                        )
                        for q in range(2):
                            dst_plane = anat if q == 0 else nat[q - 1]
                            d5 = dst_plane[:].rearrange(
                                "(qt bp h k) j -> qt bp h k j",
                                qt=4, h=2, k=16,
                            )
                            for h in range(2):
                                eng = nc.sync if h == 0 else nc.scalar
                                eng.dma_start(
                                    out=d5[qt, :, h], in_=pk5[:, h, q]
                                )

            # =========== Phases 2-4: three chained GEMMs ===========
            with (
                tc.tile_pool(name="big", bufs=1) as bigpool,
                tc.tile_pool(name="gw", bufs=3) as gpool,
                tc.tile_pool(name="nrm", bufs=4) as npool,
                tc.tile_pool(name="gps", bufs=2, space="PSUM") as gpsum,
            ):
                mv = [
                    bigpool.tile([P, 16, N], bf16, tag="mv0", name="mva"),
                    bigpool.tile([P, 16, N], bf16, tag="mv1", name="mvb"),
                ]

                # Build mv0 = a^T chunks by PE-transposing anat blocks
                anat_v = anat[:].rearrange("(ib p) k -> p ib k", p=P)
                for kc in range(16):
                    ld = gpool.tile([P, 16, P], bf16, tag="ld")
                    nc.sync.dma_start(
                        out=ld[:], in_=anat_v[:, :, kc * P : (kc + 1) * P]
                    )
                    for ib4 in range(4):
                        tp = gpsum.tile(
                            [P, 512], bf16, tag=f"ps{ib4 % 2}", name="tp"
                        )
                        for g in range(4):
                            nc.tensor.transpose(
                                tp[:, g * P : (g + 1) * P],
                                ld[:, ib4 * 4 + g, :],
                                identb[:],
                            )
                        if ib4 % 2 == 0:
                            nc.vector.tensor_copy(
                                out=mv[0][:, kc, ib4 * 512 : (ib4 + 1) * 512],
                                in_=tp[:],
                            )
                        else:
                            nc.scalar.copy(
                                mv[0][:, kc, ib4 * 512 : (ib4 + 1) * 512],
                                tp[:],
                            )

                def gemm(qi, rhs_res, out_res, normalize):
                    """Transposed-chain GEMM: out = mix_q^T @ rhs.

                    qi: q index in packed (1=b, 2=g1, 3=g2).
                    rhs_res: SBUF-resident moving operand [P, 16, N] bf16.
                    out_res: SBUF [P, 16, N] bf16 (normalize) or None (evict
                        f32 to h2t).
                    """
                    for ms in range(16):
                        bts = gpool.tile([P, 16, P], bf16, tag="bts")
                        nc.sync.dma_start(
                            out=bts[:],
                            in_=nat[qi - 1][:].rearrange(
                                "(kc p) j -> p kc j", p=P
                            )[:, :, ms * P : (ms + 1) * P],
                        )
                        ps = [
                            gpsum.tile(
                                [P, 512], f32, tag=f"ps{ic}", name=f"ps{ic}"
                            )
                            for ic in range(4)
                        ]
                        for kc in range(16):
                            for ic in range(4):
                                nc.tensor.matmul(
                                    ps[ic][:],
                                    lhsT=bts[:, kc, :],
                                    rhs=rhs_res[:, kc, ic * 512 : (ic + 1) * 512],
                                    start=(kc == 0),
                                    stop=(kc == 15),
                                )
                        if normalize:
                            dc = (ms * P) // 512
                            v = ms % 4
                            degp = npool.tile([P, 4], f32, tag="degp")
                            # zero the diagonal in place + row-sum of masked tile
                            nc.vector.scalar_tensor_tensor(
                                out=ps[dc][:],
                                in0=ps[dc][:],
                                scalar=1.0,
                                in1=masks[:, v],
                                op0=MUL,
                                op1=MUL,
                                accum_out=degp[:, dc : dc + 1],
                            )
                            for ic in range(4):
                                if ic != dc:
                                    nc.vector.tensor_reduce(
                                        degp[:, ic : ic + 1], ps[ic][:], AX, ADD
                                    )
                            degs = npool.tile([P, 1], f32, tag="degs")
                            nc.vector.tensor_reduce(degs[:], degp[:], AX, ADD)
                            dinv = npool.tile([P, 1], f32, tag="dinv")
                            nc.vector.reciprocal(dinv[:], degs[:])
                            for ic in range(4):
                                nc.scalar.activation(
                                    out_res[:, ms, ic * 512 : (ic + 1) * 512],
                                    ps[ic][:],
                                    COPY,
                                    scale=dinv[:],
                                )
                        else:
                            for ic in range(4):
                                st = gpool.tile([P, 512], f32, tag="fstage")
                                nc.scalar.copy(st[:], ps[ic][:])
                                nc.scalar.dma_start(
                                    out=h2t[ms // 4][
                                        (ms % 4) * P : (ms % 4 + 1) * P,
                                        ic * 512 : (ic + 1) * 512,
                                    ],
                                    in_=st[:],
                                )

                # GEMM1: Ht = b^T a^T ; normalize -> Hnt in mv[1]
                gemm(1, mv[0], mv[1], normalize=True)

                # unpack g1/g2 now - overlaps GEMM1/2 compute (HBM is idle)
                for q in range(2, 4):
                    d5 = nat[q - 1][:].rearrange(
                        "(qt bp h k) j -> qt bp h k j", qt=4, h=2, k=16
                    )
                    for qt in range(4):
                        pk5l = packed[qt][:].rearrange(
                            "(bp h q k) j -> bp h q k j", h=2, q=4, k=16
                        )
                        for h in range(2):
                            nc.gpsimd.dma_start(
                                out=d5[qt, :, h], in_=pk5l[:, h, q]
                            )
                # GEMM2: H't = g1^T Hnt ; normalize -> H'nt (reuse mv0 slot)
                mv0b = bigpool.tile([P, 16, N], bf16, tag="mv0")
                gemm(2, mv[1], mv0b, normalize=True)
                # GEMM3: H''t = g2^T H'nt -> h2t (f32), g2 pre-scaled by 1/16
                gemm(3, mv0b, None, normalize=False)

                # ===== Phase 5: banded AllReduce, pipelined with GEMM3 =====
                for b in range(4):
                    nc.gpsimd.collective_compute(
                        "AllReduce",
                        ADD,
                        replica_groups=[list(range(NCORES))],
                        ins=[h2t[b].opt()],
                        outs=[s_sh[b].opt()],
                    )

                # ===== Phase 6: symmetrize out = S + S^T (banded) =====
                s_cols = [
                    s_sh[b][:].rearrange("(nb p) m -> p nb m", p=P)
                    for b in range(4)
                ]
                for ms in range(16):
                    srow = gpool.tile([P, N], f32, tag="srow", bufs=2)
                    nc.sync.dma_start(
                        out=srow[:],
                        in_=s_sh[ms // 4][(ms % 4) * P : (ms % 4 + 1) * P, :],
                    )
                    ost = gpool.tile([P, N], f32, tag="ost", bufs=2)
                    for b in range(4):
                        colb = gpool.tile([P, 4, P], f32, tag="colb")
                        nc.sync.dma_start(
                            out=colb[:],
                            in_=s_cols[b][:, :, ms * P : (ms + 1) * P],
                        )
                        pst = gpsum.tile(
                            [P, 512], f32, tag=f"ps{b % 2}", name="pst"
                        )
                        for g in range(4):
                            nc.tensor.transpose(
                                pst[:, g * P : (g + 1) * P],
                                colb[:, g, :],
                                ident[:],
                            )
                        nc.vector.scalar_tensor_tensor(
                            out=ost[:, b * 512 : (b + 1) * 512],
                            in0=srow[:, b * 512 : (b + 1) * 512],
                            scalar=1.0,
                            in1=pst[:],
                            op0=MUL,
                            op1=ADD,
                        )
                    nc.scalar.dma_start(
                        out=out_ext[ms * P : (ms + 1) * P, :], in_=ost[:]
                    )


    nc.compile()
    return nc


def _get_program():
    global _PROGRAM
    if _PROGRAM is None:
        _PROGRAM = _build_program()
    return _PROGRAM


def _make_wblk(sws) -> np.ndarray:
    """Block-diagonal mix weights [128, 16*len(sws)].

    wblk[(x*8+e), (q*16+x)] = sws[q][e]  for x in 0..15.
    Partitions = (16 x, 8 e) matching the host-permuted A layout; out
    partitions = (q, 16 x).
    """
    wblk = np.zeros((P, 16 * len(sws)), np.float32)
    for q, sw in enumerate(sws):
        for x in range(16):
            wblk[x * 8 : (x + 1) * 8, q * 16 + x] = sw.astype(np.float32)
    return wblk


def _prep_inputs(A, w1_0, w2_0, w_1, w_2):
    import ml_dtypes

    swa = _softmax_rows(np.asarray(w1_0))
    swb = _softmax_rows(np.asarray(w2_0))
    sg1 = _softmax_rows(np.asarray(w_1))
    # fold mean over channels (1/8) and symmetrize (1/2) into the last mix
    sg2 = _softmax_rows(np.asarray(w_2)) / 16.0

    abf = np.asarray(A, dtype=np.float32)[0].astype(ml_dtypes.bfloat16)  # [k,j,e]
    # At3[b, (k16 e), j] = A[16b+k16, j, e]
    at3 = np.ascontiguousarray(abf.transpose(0, 2, 1).reshape(P, P, N))
    in_maps = []
    for c in range(NCORES):
        w4 = _make_wblk([swa[c], swb[c], sg1[c], sg2[c]]).astype(
            ml_dtypes.bfloat16
        )
        in_maps.append({"At3": at3, "wblk4": w4})
    return in_maps


def kernel(A, w1_0, w2_0, w_1, w_2):
    from concourse.bass_utils import run_bass_kernel_spmd

    in_maps = _prep_inputs(A, w1_0, w2_0, w_1, w_2)
    nc = _get_program()
    res = run_bass_kernel_spmd(nc, in_maps, list(range(NCORES)))
    return np.asarray(res.results[0]["out"], dtype=np.float32)



# revision 10
# speedup vs baseline: 2.3438x; 2.3438x over previous
"""GTN (graph transformer network) meta-path kernel for TRN2, 8 NeuronCores.

Math (reference):
    Ap = A transposed to [E, N, N]
    a  = sum_e softmax(w1_0)[c,e] * Ap[e]      (per channel c)
    b  = sum_e softmax(w2_0)[c,e] * Ap[e]
    H  = a @ b
    twice:  H = normalize(H) @ gtconv(Ap, w)   (normalize = zero diag, col-scale)
    out = symmetrized mean over channels.

Sharding: channel-parallel — core c computes channel c end to end (the four
softmax mixes differ only in the tiny [E] weight vector, fed per-core).  Each
core locally symmetrizes G_c = (H''_c + H''_c^T)/16, one ReduceScatter sums
the G_c and leaves each core a 256-row band of the result; the host stacks
the 8 bands.

All heavy compute runs in fp8 (e4m3):
 - A is uploaded pre-permuted fp8; one DoubleRow PE pass computes all four
   mixes straight into natural-layout DRAM (no packed/unpack round trip).
 - The three chained 2048^3 GEMMs run fp8 DoubleRow (2 k-tiles per pass).
   The normalized intermediates are scaled by S=1024 so their ~1/N entries
   sit near 0.5 where e4m3 has full precision; S cancels inside normalize
   and 1/(16*S) is folded into the GEMM3 eviction scale.

On-device formulation works with TRANSPOSED intermediates Ht = H^T so that
every GEMM's moving operand is the previous GEMM's output as-is, and
normalization becomes row sums (free-dim reduce) + per-partition scale.
"""

import numpy as np

N = 2048
E = 8
C = 8
P = 128
NCORES = 8
S_SCALE = 1024.0

_PROGRAM = None


def _softmax_rows(w: np.ndarray) -> np.ndarray:
    """w: [C, E, 1, 1] -> softmax over E, float64 precision, returns [C, E]."""
    x = w.reshape(C, E).astype(np.float64)
    x = x - x.max(axis=1, keepdims=True)
    ex = np.exp(x)
    return ex / ex.sum(axis=1, keepdims=True)


def _build_program():
    import concourse.bacc as bacc
    import concourse.mybir as mybir
    import concourse.tile as tile
    from concourse.masks import make_identity

    f32 = mybir.dt.float32
    bf16 = mybir.dt.bfloat16
    fp8 = mybir.dt.float8e4
    AX = mybir.AxisListType.X
    MUL = mybir.AluOpType.mult
    ADD = mybir.AluOpType.add
    NE = mybir.AluOpType.not_equal
    COPY = mybir.ActivationFunctionType.Copy
    RECIP = mybir.ActivationFunctionType.Reciprocal
    DR = mybir.MatmulPerfMode.DoubleRow

    nc = bacc.Bacc("TRN2")
    A3_ext = nc.dram_tensor("At3", [P, P, N], fp8, kind="ExternalInput")
    w4_ext = nc.dram_tensor("w4d", [P, 2, P], fp8, kind="ExternalInput")
    out_ext = nc.dram_tensor("out", [N // NCORES, N], f32, kind="ExternalOutput")

    with tile.TileContext(nc) as tc:
        with (
            tc.tile_pool(name="dram", bufs=1, space="DRAM") as dpool,
            tc.tile_pool(name="const", bufs=1) as cpool,
        ):
            # the four mixes in natural [i, j] layout (a, b, g1, g2)
            nat = [dpool.tile([N, N], fp8, name=f"nat{q}") for q in range(4)]
            gsym = dpool.tile([N, N], bf16, name="gsym")
            g_sh = dpool.tile([N // NCORES, N], bf16, name="gsh")

            # --- constants ---
            w4_sb = cpool.tile([P, 2, P], fp8)
            nc.sync.dma_start(out=w4_sb[:], in_=w4_ext[:])
            ident8 = cpool.tile([P, P], fp8)
            make_identity(nc, ident8[:])
            identb = cpool.tile([P, P], bf16)
            make_identity(nc, identb[:])
            # diag masks: masks[:, v, y] = 0 where y == p + v*128 else 1
            masks = cpool.tile([P, 4, 512], f32)
            nc.gpsimd.memset(masks[:], 1.0)
            for v in range(4):
                nc.gpsimd.affine_select(
                    out=masks[:, v],
                    in_=masks[:, v],
                    compare_op=NE,
                    fill=0.0,
                    base=v * P,
                    pattern=[[-1, 512]],
                    channel_multiplier=1,
                )

            # ======== Phase 1: all four mixes, one DoubleRow PE pass ========
            # a3t partitions hold (k16, e); the duplicated block-diag weight
            # computes two row blocks (h) of all four mixes per matmul:
            # pm row (h, q, x) = mix_q[16*(4*ld4 + 2*half + h) + x, :].
            with (
                tc.tile_pool(name="mix", bufs=3) as mpool,
                tc.tile_pool(name="mixst", bufs=3) as spool,
                tc.tile_pool(name="mixps", bufs=6, space="PSUM") as mpsum,
            ):
                for ld4 in range(32):
                    a3t = mpool.tile([P, 4, N], fp8, tag="a3t")
                    nc.sync.dma_start(
                        out=a3t[:],
                        in_=A3_ext[4 * ld4 : 4 * ld4 + 4].rearrange(
                            "b p j -> p b j"
                        ),
                    )
                    stg = spool.tile([P, 2, N], fp8, tag="stg")
                    for half in range(2):
                        for jc in range(4):
                            pm = mpsum.tile([P, 512], f32, tag="pm")
                            nc.tensor.matmul(
                                pm[:],
                                lhsT=w4_sb[:],
                                rhs=a3t[
                                    :,
                                    2 * half : 2 * half + 2,
                                    jc * 512 : (jc + 1) * 512,
                                ],
                                start=True,
                                stop=True,
                                perf_mode=DR,
                            )
                            dst = stg[:, half, jc * 512 : (jc + 1) * 512]
                            if jc % 2 == 0:
                                nc.vector.tensor_copy(out=dst, in_=pm[:])
                            else:
                                nc.scalar.copy(dst, pm[:])
                    # natural-layout row band: rows 64*ld4 .. 64*ld4+64 of
                    # each mix; one DMA per (q, h), 3-dim APs on both sides
                    for q in range(4):
                        for h in range(2):
                            weng = (nc.gpsimd, nc.scalar, nc.sync)[
                                (2 * q + h) % 3
                            ]
                            weng.dma_start(
                                out=nat[q][
                                    64 * ld4 : 64 * ld4 + 64, :
                                ].rearrange(
                                    "(half h x) j -> h x half j", half=2, h=2
                                )[h],
                                in_=stg[h * 64 + q * 16 : h * 64 + q * 16 + 16],
                            )

            # =========== Phases 2-4: three chained GEMMs (fp8 DR) ===========
            with (
                tc.tile_pool(name="big", bufs=1) as bigpool,
                tc.tile_pool(name="gw", bufs=3) as gpool,
                tc.tile_pool(name="nrm", bufs=4) as npool,
            ):
                mv = [
                    bigpool.tile([P, 16, N], fp8, tag="mv0", name="mva"),
                    bigpool.tile([P, 16, N], fp8, tag="mv1", name="mvb"),
                ]
                h2t_sb = bigpool.tile([P, 16, N], bf16, tag="h2t", name="h2t")

                # Build mv0 = a^T chunks by PE-transposing nat0 blocks (fp8)
                anat_v = nat[0][:].rearrange("(ib p) k -> p ib k", p=P)
                with tc.tile_pool(name="tps", bufs=2, space="PSUM") as tpsum:
                    for kc in range(16):
                        ld = gpool.tile([P, 16, P], fp8, tag="ld")
                        nc.sync.dma_start(
                            out=ld[:], in_=anat_v[:, :, kc * P : (kc + 1) * P]
                        )
                        for ib4 in range(4):
                            # fp8 transpose writes PSUM at element step 2
                            tp = tpsum.tile(
                                [P, 512, 2], fp8, tag=f"tp{ib4 % 2}", name="tp"
                            )
                            for g in range(4):
                                nc.tensor.transpose(
                                    tp[:, g * P : (g + 1) * P, 0],
                                    ld[:, ib4 * 4 + g, :],
                                    ident8[:],
                                )
                            if ib4 % 2 == 0:
                                nc.vector.tensor_copy(
                                    out=mv[0][
                                        :, kc, ib4 * 512 : (ib4 + 1) * 512
                                    ],
                                    in_=tp[:, :, 0],
                                )
                            else:
                                nc.scalar.copy(
                                    mv[0][:, kc, ib4 * 512 : (ib4 + 1) * 512],
                                    tp[:, :, 0],
                                )

                def gemm(qi, rhs_res, out_res, gpsum):
                    """Transposed-chain GEMM: out = mix_q^T @ rhs, fp8 DR.

                    qi: q index in nat (1=b, 2=g1, 3=g2).
                    rhs_res: SBUF-resident moving operand [P, 16, N] fp8,
                        holding S * (previous normalized intermediate)^T.
                    out_res: SBUF [P, 16, N] fp8 -> normalize, evict with
                        dinv*S; None -> evict bf16 to h2t_sb with 1/(16*S).
                    """
                    for ms in range(16):
                        bts = gpool.tile([P, 16, P], fp8, tag="bts")
                        nc.sync.dma_start(
                            out=bts[:],
                            in_=nat[qi][:].rearrange(
                                "(kc p) j -> p kc j", p=P
                            )[:, :, ms * P : (ms + 1) * P],
                        )
                        ps = [
                            gpsum.tile(
                                [P, 512], f32, tag=f"ps{ic}", name=f"ps{ic}"
                            )
                            for ic in range(4)
                        ]
                        for kc2 in range(8):
                            for ic in range(4):
                                nc.tensor.matmul(
                                    ps[ic][:],
                                    lhsT=bts[:, 2 * kc2 : 2 * kc2 + 2, :],
                                    rhs=rhs_res[
                                        :,
                                        2 * kc2 : 2 * kc2 + 2,
                                        ic * 512 : (ic + 1) * 512,
                                    ],
                                    start=(kc2 == 0),
                                    stop=(kc2 == 7),
                                    perf_mode=DR,
                                )
                        if out_res is not None:
                            dc = (ms * P) // 512
                            v = ms % 4
                            degp = npool.tile([P, 4], f32, tag="degp")
                            # zero the diagonal in place + row-sum of masked
                            nc.vector.scalar_tensor_tensor(
                                out=ps[dc][:],
                                in0=ps[dc][:],
                                scalar=1.0,
                                in1=masks[:, v],
                                op0=MUL,
                                op1=MUL,
                                accum_out=degp[:, dc : dc + 1],
                            )
                            for ic in range(4):
                                if ic != dc:
                                    nc.vector.tensor_reduce(
                                        degp[:, ic : ic + 1], ps[ic][:], AX, ADD
                                    )
                            degs = npool.tile([P, 1], f32, tag="degs")
                            nc.vector.tensor_reduce(degs[:], degp[:], AX, ADD)
                            # dinv = S/deg, so the fp8 store sits near 0.5
                            degss = npool.tile([P, 1], f32, tag="degss")
                            nc.vector.tensor_scalar_mul(
                                degss[:], degs[:], 1.0 / S_SCALE
                            )
                            dinv = npool.tile([P, 1], f32, tag="dinv")
                            nc.vector.reciprocal(dinv[:], degss[:])
                            for ic in range(4):
                                nc.scalar.activation(
                                    out_res[:, ms, ic * 512 : (ic + 1) * 512],
                                    ps[ic][:],
                                    COPY,
                                    scale=dinv[:],
                                )
                        else:
                            for ic in range(4):
                                nc.scalar.activation(
                                    h2t_sb[:, ms, ic * 512 : (ic + 1) * 512],
                                    ps[ic][:],
                                    COPY,
                                    scale=1.0 / (16.0 * S_SCALE),
                                )

                with tc.tile_pool(name="gps", bufs=2, space="PSUM") as gpsum:
                    # GEMM1: Ht = b^T a^T ; normalize -> S*Hnt in mv[1]
                    gemm(1, mv[0], mv[1], gpsum)
                    # GEMM2: H't = g1^T (S Hnt) ; normalize -> S*H'nt
                    mv0b = bigpool.tile([P, 16, N], fp8, tag="mv0")
                    gemm(2, mv[1], mv0b, gpsum)
                    # GEMM3: H''t/16 = g2^T (S H'nt)/(16 S) -> h2t_sb (bf16)
                    gemm(3, mv0b, None, gpsum)

                # ===== Phase 5: local symmetrize G = (H'' + H''^T)/16 =====
                with tc.tile_pool(name="sps", bufs=2, space="PSUM") as spsum:
                    for ms in range(16):
                        tps = [
                            spsum.tile(
                                [P, 512], bf16, tag=f"sp{b4}", name="tps"
                            )
                            for b4 in range(4)
                        ]
                        for kc in range(16):
                            nc.tensor.transpose(
                                tps[kc // 4][
                                    :, (kc % 4) * P : (kc % 4 + 1) * P
                                ],
                                h2t_sb[:, kc, ms * P : (ms + 1) * P],
                                identb[:],
                            )
                        gst = gpool.tile([P, N], bf16, tag="gst")
                        for b4 in range(4):
                            nc.vector.scalar_tensor_tensor(
                                out=gst[:, b4 * 512 : (b4 + 1) * 512],
                                in0=h2t_sb[:, ms, b4 * 512 : (b4 + 1) * 512],
                                scalar=1.0,
                                in1=tps[b4][:],
                                op0=MUL,
                                op1=ADD,
                            )
                        weng = nc.scalar if ms % 2 else nc.sync
                        weng.dma_start(
                            out=gsym[ms * P : (ms + 1) * P, :], in_=gst[:]
                        )

                # ===== Phase 6: ReduceScatter -> this core's 256-row band ===
                nc.gpsimd.collective_compute(
                    "ReduceScatter",
                    ADD,
                    replica_groups=[list(range(NCORES))],
                    ins=[gsym[:].opt()],
                    outs=[g_sh[:].opt()],
                )

                # ===== Phase 7: upcast band to f32 output =====
                for r in range(2):
                    gb = gpool.tile([P, N], bf16, tag="gst")
                    nc.sync.dma_start(
                        out=gb[:], in_=g_sh[r * P : (r + 1) * P, :]
                    )
                    ob = gpool.tile([P, N], f32, tag="ob", bufs=2)
                    nc.vector.tensor_copy(out=ob[:], in_=gb[:])
                    nc.scalar.dma_start(
                        out=out_ext[r * P : (r + 1) * P, :], in_=ob[:]
                    )

    nc.compile()
    return nc


def _get_program():
    global _PROGRAM
    if _PROGRAM is None:
        _PROGRAM = _build_program()
    return _PROGRAM


def _make_w4d(sws) -> np.ndarray:
    """Duplicated block-diagonal mix weights [128, 2, 128].

    w4d[(x*8+e), h, h2*64 + q*16 + xx] = sws[q][e] if (h==h2 and x==xx).
    With DoubleRow the two k-tiles (h) of the moving operand produce the
    two row blocks h2 of the packed (q, x) mix outputs.
    """
    w4d = np.zeros((P, 2, P), np.float32)
    for q, sw in enumerate(sws):
        for x in range(16):
            for h in range(2):
                w4d[x * 8 : (x + 1) * 8, h, h * 64 + q * 16 + x] = sw.astype(
                    np.float32
                )
    return w4d


def _prep_inputs(A, w1_0, w2_0, w_1, w_2):
    import ml_dtypes

    e4 = ml_dtypes.float8_e4m3
    swa = _softmax_rows(np.asarray(w1_0))
    swb = _softmax_rows(np.asarray(w2_0))
    sg1 = _softmax_rows(np.asarray(w_1))
    sg2 = _softmax_rows(np.asarray(w_2))

    a8 = np.asarray(A, dtype=np.float32)[0].astype(e4)  # [k, j, e]
    # At3[kb, (k16 e), j] = A[16*kb + k16, j, e]
    at3 = np.ascontiguousarray(a8.transpose(0, 2, 1).reshape(P, P, N))
    in_maps = []
    for c in range(NCORES):
        w4d = _make_w4d([swa[c], swb[c], sg1[c], sg2[c]]).astype(e4)
        in_maps.append({"At3": at3, "w4d": w4d})
    return in_maps


def kernel(A, w1_0, w2_0, w_1, w_2):
    from concourse.bass_utils import run_bass_kernel_spmd

    in_maps = _prep_inputs(A, w1_0, w2_0, w_1, w_2)
    nc = _get_program()
    res = run_bass_kernel_spmd(nc, in_maps, list(range(NCORES)))
    return np.concatenate(
        [np.asarray(res.results[k]["out"], dtype=np.float32) for k in range(NCORES)],
        axis=0,
    )
